# revision 12
# baseline (speedup 1.0000x reference)
"""KAN layer kernel for Trainium2, data-parallel over 8 NeuronCores.

Math: out[b,o] = sum_i comb_w[i,o] * (w1*x + w2*x^2 + w3*x^3 + edge_b)[b,i,o]
    = x @ W1 + x^2 @ W2 + x^3 @ W3 + bias
  where Wp[i,o] = edge_w[i,o,p] * comb_w[i,o],  bias[o] = sum_i comb_w[i,o]*edge_b[i,o].

Stacked along the contraction dim this is one [B,1536] @ [1536,512] matmul.

Sharding: batch 8-way (1024 rows/core), weights replicated. Everything is
bf16 on the wire and in the matmuls (fp32 PSUM accumulation): bf16 runs the
PE at full rate (1 cyc/row vs ~2 for fp32r on HW) and halves DMA bytes.
Verified numerics: max rel err ~6e-3 vs the fp32 reference (gate is 2e-2).

Per core:
- x^T arrives as [512, 1024] bf16; weights as 13 chunks of [128, 512] bf16
  in consumption order (chunk 3t+p = basis power p of k-tile t; chunk 12
  carries the fp32 bias bit-packed into bf16 pairs).
- DMA is split across both HWDGE queues: x + output on the sync queue,
  weights + bias on the scalar queue. x and w are fetched per k-tile so
  matmuls start after the first tile lands, not after the whole tensor.
- DVE computes x^2/x^3 per tile as it arrives.
- 48 matmuls of [128k,128o] x [128k, 1024b] (bf16 moving max), PSUM as
  4 tiles of [128, 1024] f32 = all 8 banks; accumulate 12 chunks each.
  Phase A (o-tiles 0,1) is t-major to pipeline with the x DMA; phase B
  (o-tiles 2,3) is o-major so each o-tile's output drains (bias add on
  DVE + DMA out) while the next one is still accumulating.
- Output is written as y^T [512, 1024] bf16; host transposes/casts back.
"""

import sys

import numpy as np
import ml_dtypes

sys.path.insert(0, "/opt/trn_rl_repo")

import concourse.bass as bass
import concourse.tile as tile
from concourse import bass_utils, mybir
from concourse.tile_rust import add_dep_helper

B, I, O = 8192, 512, 512
NCORES = 8
BS = B // NCORES  # 1024 rows per core
PT = 4  # 128-row tiles in I (k-tiles) and O (o-tiles)
NPOW = 3  # basis powers: x, x^2, x^3
NCHUNK = NPOW * PT  # 12 contraction chunks of 128
WROWS = (NCHUNK + 1) * 128  # 12 weight chunks + 1 bias chunk

BF = mybir.dt.bfloat16
F32 = mybir.dt.float32

_nc = None


def _build():
    # All HBM tensors are partition-major (leading dim 128 = SBUF partition)
    # so each DMA descriptor covers one partition's full contiguous span
    # (2-6 KB) instead of a 1 KB line — the HWDGE descriptor generator is
    # the head-latency bottleneck at ~20-25 ns/descriptor.
    nc = bass.Bass("TRN2", target_bir_lowering=False, debug=False)
    xt = nc.dram_tensor("xt", [128, PT, BS], BF, kind="ExternalInput")
    w = nc.dram_tensor("w", [128, NCHUNK + 1, O], BF, kind="ExternalInput")
    yt = nc.dram_tensor("yt", [128, PT, BS], BF, kind="ExternalOutput")

    xt_r = xt.ap()  # [128, 4, 1024]
    w_r = w.ap()  # [128, 13, 512]
    yt_r = yt.ap()  # [128, 4, 1024]

    pe_chain = []  # forced PE program order (sync=False edges)

    def pe(inst):
        if pe_chain:
            add_dep_helper(inst.ins, pe_chain[-1].ins, sync=False, reason="pe order")
        pe_chain.append(inst)
        return inst

    # HAM warm-up: ~3.4us of dummy matmuls on garbage SBUF in the main
    # block, so the PE clock gate is already at 8/8 (2.4 GHz) when the
    # real matmuls start. The scratch PSUM bank is freed before the tile
    # pools allocate; real banks are zeroed by their start=True matmuls.
    warm_w = nc.alloc_sbuf_tensor("warm_w", [128, 2], BF)
    warm_x = nc.alloc_sbuf_tensor("warm_x", [128, 512], BF)
    with nc.psum_tensor("warm_ps", [128, 512], F32) as wps:
        for i in range(8):
            nc.tensor.matmul(
                wps.ap()[0:2, :], warm_w.ap(), warm_x.ap(), start=True, stop=True
            )
    # DMA warm-up: the DMA subsystem also ramps (~40 GB/s cold ->
    # 130-250 GB/s after a few us of activity). Fire-and-forget reads on
    # both HWDGE queues from the main block so the queues are warm when
    # the real transfers issue. Results land in scratch; no sync needed.
    warm_d = nc.alloc_sbuf_tensor("warm_d", [128, 2, 256], BF)
    warm_sem = nc.alloc_semaphore("warm_dma_sem")
    for i in range(2):
        nc.sync.dma_start(out=warm_d.ap()[:, 0, :], in_=xt.ap()[:, 0, 0:256]).then_inc(
            warm_sem, 16
        )
        nc.scalar.dma_start(
            out=warm_d.ap()[:, 1, :], in_=w.ap()[:, 0, 0:256]
        ).then_inc(warm_sem, 16)

    with tile.TileContext(nc) as tc:
        with (
            tc.tile_pool(name="consts", bufs=1) as cpool,
            tc.tile_pool(name="acts", bufs=1) as apool,
            tc.tile_pool(name="out", bufs=1) as opool,
            tc.tile_pool(name="psum", bufs=1, space="PSUM") as pspool,
        ):
            w_sb = cpool.tile([128, NCHUNK + 1, O], BF)
            x_sb = apool.tile([128, PT, BS], BF)
            x2_sb = apool.tile([128, PT, BS], BF)
            x3_sb = apool.tile([128, PT, BS], BF)
            y_sb = opool.tile([128, PT, BS], BF)

            # bias: chunk 12, cols 0..7 hold [128,4] f32 bit-packed as bf16 pairs
            bias_f32 = w_sb[:, NCHUNK, 0:8].bitcast(F32)  # [128, 4]

            # x on the sync queue: phase A's operands (batch half 0) first,
            # smallest piece leading so matmuls start during the DMA ramp;
            # all of half 1 as one piece behind. DVE squares/cubes each
            # (tile, half) as it lands. Weights on the scalar queue in
            # consumption order, single chunks while the queue is cold.
            h0, h1 = slice(0, 512), slice(512, 1024)
            nc.sync.dma_start(out=x_sb[:, 0:1, h0], in_=xt_r[:, 0:1, h0])
            nc.sync.dma_start(out=x_sb[:, 1:PT, h0], in_=xt_r[:, 1:PT, h0])
            nc.sync.dma_start(out=x_sb[:, :, h1], in_=xt_r[:, :, h1])
            for h in (h0, h1):
                for t in range(PT):
                    nc.vector.tensor_mul(
                        x2_sb[:, t, h], x_sb[:, t, h], x_sb[:, t, h]
                    )
                    nc.vector.tensor_mul(
                        x3_sb[:, t, h], x2_sb[:, t, h], x_sb[:, t, h]
                    )
            for sl in (
                slice(0, 1),
                slice(1, 2),
                slice(2, 3),
                slice(3, 9),
                slice(9, NCHUNK + 1),
            ):
                nc.scalar.dma_start(out=w_sb[:, sl, :], in_=w_r[:, sl, :])

            basis = [x_sb, x2_sb, x3_sb]
            # 8 PSUM banks: ps[n*4+o] = batch half n, o-tile o, [128, 512] f32
            ps = [
                pspool.tile([128, 512], F32, name=f"ps{i}", tag=f"ps{i}")
                for i in range(2 * PT)
            ]

            def mm(n, o, t, p):
                pe(
                    nc.tensor.matmul(
                        ps[n * PT + o],
                        w_sb[:, 3 * t + p, o * 128 : (o + 1) * 128],
                        basis[p][:, t, n * 512 : (n + 1) * 512],
                        start=(t == 0 and p == 0),
                        stop=(t == PT - 1 and p == NPOW - 1),
                    )
                )

            def copy_out(n, o):
                # PSUM -> SBUF with bias add (f32 -> bf16)
                nc.vector.tensor_scalar_add(
                    y_sb[:, o, n * 512 : (n + 1) * 512],
                    ps[n * PT + o],
                    bias_f32[:, o : o + 1],
                )

            # phase A: batch half 0, t-major (pipelines with x arrival)
            for t in range(PT):
                for p in range(NPOW):
                    for o in range(PT):
                        mm(0, o, t, p)
            # phase B: batch half 1, o-major (early per-o drain).
            # half-0 copies run on DVE as soon as phase A's banks stop.
            for o in range(PT):
                copy_out(0, o)
            for o in range(PT):
                for t in range(PT):
                    for p in range(NPOW):
                        mm(1, o, t, p)
                copy_out(1, o)
                # both halves of o-tile o are in y_sb now; ship it
                queue = nc.scalar if o % 2 == 0 else nc.sync
                queue.dma_start(out=yt_r[:, o, :], in_=y_sb[:, o, :])

    # Post-pass: walrus codegen admits only one sync-wait per instruction
    # encoding here; Tile's kernel-tail drain aggregates one wait per
    # outstanding semaphore. Split any multi-wait instruction into a chain
    # of single-wait drains ahead of it on the same engine queue.
    for bb in nc.m.functions[0].blocks:
        insts = list(bb.instructions)
        out, split = [], 0
        for ins in insts:
            si = ins.sync_info
            waits = list(si.on_wait) if si and si.on_wait else []
            if len(waits) > 1:
                for wx in waits[:-1]:
                    nd = mybir.InstDrain(
                        name=f"drain_split_{split}", engine=ins.engine
                    )
                    split += 1
                    nd.sync_info = mybir.SyncInfo(on_wait=[wx], on_update=[])
                    out.append(nd)
                si.on_wait = [waits[-1]]
            out.append(ins)
        if split:
            bb.set_instructions_from_list(out) if hasattr(
                bb, "set_instructions_from_list"
            ) else setattr(bb, "instructions", out)
    return nc


last_results = None  # BassKernelResults of the most recent run (for test harness)


def kernel(x, edge_w, edge_b, comb_w):
    global _nc, last_results
    if _nc is None:
        _nc = _build()

    bf16 = ml_dtypes.bfloat16
    w_eff = (edge_w * comb_w[:, :, None]).astype(np.float32)  # [I, O, 3]
    # chunk c = 3t+p: rows 128t..128(t+1) of W_p, in matmul consumption order
    w_big = np.empty((NCHUNK + 1, 128, O), dtype=bf16)
    for t in range(PT):
        for p in range(NPOW):
            w_big[3 * t + p] = w_eff[t * 128 : (t + 1) * 128, :, p].astype(bf16)
    # bias chunk: [128,4] f32 bit-packed into bf16 pairs at cols 0..7
    bias = np.sum(comb_w * edge_b, axis=0, dtype=np.float64).astype(np.float32)
    pad = np.zeros((128, O), dtype=bf16)
    pad_u16 = pad.view(np.uint16)
    pad_u16[:, :8] = np.ascontiguousarray(bias.reshape(PT, 128).T).view(np.uint16)
    w_big[NCHUNK] = pad
    # partition-major: [128, 13, 512]
    w_pm = np.ascontiguousarray(w_big.transpose(1, 0, 2))

    in_maps = []
    for c in range(NCORES):
        xs = x[c * BS : (c + 1) * BS].T.astype(bf16)  # [I, BS]
        # partition-major: [128, 4, 1024], [p, t, b] = x^T[t*128+p, b]
        xs_pm = np.ascontiguousarray(xs.reshape(PT, 128, BS).transpose(1, 0, 2))
        in_maps.append({"xt": xs_pm, "w": w_pm})

    res = bass_utils.run_bass_kernel_spmd(_nc, in_maps, list(range(NCORES)))
    last_results = res
    outs = []
    for c in range(NCORES):
        yt = np.asarray(res.results[c]["yt"])  # [128, 4, 1024] bf16
        # un-permute to y^T [512, 1024], then transpose to [1024, 512]
        outs.append(yt.transpose(1, 0, 2).reshape(O, BS).T.astype(np.float32))
    return np.concatenate(outs, axis=0)


# revision 14
# speedup vs baseline: 1.0571x; 1.0571x over previous
"""KAN layer kernel for Trainium2, data-parallel over 8 NeuronCores.

Math: out[b,o] = sum_i comb_w[i,o] * (w1*x + w2*x^2 + w3*x^3 + edge_b)[b,i,o]
    = x @ W1 + x^2 @ W2 + x^3 @ W3 + bias
  where Wp[i,o] = edge_w[i,o,p] * comb_w[i,o],  bias[o] = sum_i comb_w[i,o]*edge_b[i,o].

Stacked along the contraction dim this is one [B,1536] @ [1536,512] matmul.

Sharding: batch 8-way (1024 rows/core), weights replicated. Everything is
bf16 on the wire and in the matmuls (fp32 PSUM accumulation): bf16 runs the
PE at full rate (1 cyc/row vs ~2 for fp32r on HW) and halves DMA bytes.
Verified numerics: max rel err ~6e-3 vs the fp32 reference (gate is 2e-2).

Per core:
- x^T arrives as [512, 1024] bf16; weights as 13 chunks of [128, 512] bf16
  in consumption order (chunk 3t+p = basis power p of k-tile t; chunk 12
  carries the fp32 bias bit-packed into bf16 pairs).
- DMA is split across both HWDGE queues: x + output on the sync queue,
  weights + bias on the scalar queue. x and w are fetched per k-tile so
  matmuls start after the first tile lands, not after the whole tensor.
- DVE computes x^2/x^3 per tile as it arrives.
- 48 matmuls of [128k,128o] x [128k, 1024b] (bf16 moving max), PSUM as
  4 tiles of [128, 1024] f32 = all 8 banks; accumulate 12 chunks each.
  Phase A (o-tiles 0,1) is t-major to pipeline with the x DMA; phase B
  (o-tiles 2,3) is o-major so each o-tile's output drains (bias add on
  DVE + DMA out) while the next one is still accumulating.
- Output is written as y^T [512, 1024] bf16; host transposes/casts back.
"""

import sys

import numpy as np
import ml_dtypes

sys.path.insert(0, "/opt/trn_rl_repo")

import concourse.bass as bass
import concourse.tile as tile
from concourse import bass_utils, mybir
from concourse.tile_rust import add_dep_helper

B, I, O = 8192, 512, 512
NCORES = 8
BS = B // NCORES  # 1024 rows per core
PT = 4  # 128-row tiles in I (k-tiles) and O (o-tiles)
NPOW = 3  # basis powers: x, x^2, x^3
NCHUNK = NPOW * PT  # 12 contraction chunks of 128
WROWS = (NCHUNK + 1) * 128  # 12 weight chunks + 1 bias chunk

BF = mybir.dt.bfloat16
F32 = mybir.dt.float32

_nc = None


def _build():
    # All HBM tensors are partition-major (leading dim 128 = SBUF partition)
    # so each DMA descriptor covers one partition's full contiguous span
    # (2-6 KB) instead of a 1 KB line — the HWDGE descriptor generator is
    # the head-latency bottleneck at ~20-25 ns/descriptor.
    nc = bass.Bass("TRN2", target_bir_lowering=False, debug=False)
    xt = nc.dram_tensor("xt", [128, PT, BS], BF, kind="ExternalInput")
    w = nc.dram_tensor("w", [128, NCHUNK + 1, O], BF, kind="ExternalInput")
    yt = nc.dram_tensor("yt", [128, PT, BS], BF, kind="ExternalOutput")

    xt_r = xt.ap()  # [128, 4, 1024]
    w_r = w.ap()  # [128, 13, 512]
    yt_r = yt.ap()  # [128, 4, 1024]

    pe_chain = []  # forced PE program order (sync=False edges)

    def pe(inst):
        if pe_chain:
            add_dep_helper(inst.ins, pe_chain[-1].ins, sync=False, reason="pe order")
        pe_chain.append(inst)
        return inst

    # HAM warm-up: ~3.4us of dummy matmuls on garbage SBUF in the main
    # block, so the PE clock gate is already at 8/8 (2.4 GHz) when the
    # real matmuls start. The scratch PSUM bank is freed before the tile
    # pools allocate; real banks are zeroed by their start=True matmuls.
    warm_w = nc.alloc_sbuf_tensor("warm_w", [128, 2], BF)
    warm_x = nc.alloc_sbuf_tensor("warm_x", [128, 512], BF)
    with nc.psum_tensor("warm_ps", [128, 512], F32) as wps:
        for i in range(8):
            nc.tensor.matmul(
                wps.ap()[0:2, :], warm_w.ap(), warm_x.ap(), start=True, stop=True
            )
    # (A DMA warm-up was tried here and hurt: the extra DMA_DIRECT2D
    # issues delay the real transfers by more than the ramp saves.)

    with tile.TileContext(nc) as tc:
        with (
            tc.tile_pool(name="consts", bufs=1) as cpool,
            tc.tile_pool(name="acts", bufs=1) as apool,
            tc.tile_pool(name="out", bufs=1) as opool,
            tc.tile_pool(name="psum", bufs=1, space="PSUM") as pspool,
        ):
            w_sb = cpool.tile([128, NCHUNK + 1, O], BF)
            x_sb = apool.tile([128, PT, BS], BF)
            x2_sb = apool.tile([128, PT, BS], BF)
            x3_sb = apool.tile([128, PT, BS], BF)
            y_sb = opool.tile([128, PT, BS], BF)

            # bias: chunk 12, cols 0..7 hold [128,4] f32 bit-packed as bf16 pairs
            bias_f32 = w_sb[:, NCHUNK, 0:8].bitcast(F32)  # [128, 4]

            # x on the sync queue: phase A's operands (batch half 0) first,
            # smallest piece leading so matmuls start during the DMA ramp;
            # all of half 1 as one piece behind. DVE squares/cubes each
            # (tile, half) as it lands. Weights on the scalar queue in
            # consumption order, single chunks while the queue is cold.
            h0, h1 = slice(0, 512), slice(512, 1024)
            nc.sync.dma_start(out=x_sb[:, 0:1, h0], in_=xt_r[:, 0:1, h0])
            nc.sync.dma_start(out=x_sb[:, 1:PT, h0], in_=xt_r[:, 1:PT, h0])
            nc.sync.dma_start(out=x_sb[:, :, h1], in_=xt_r[:, :, h1])
            for h in (h0, h1):
                for t in range(PT):
                    nc.vector.tensor_mul(
                        x2_sb[:, t, h], x_sb[:, t, h], x_sb[:, t, h]
                    )
                    nc.vector.tensor_mul(
                        x3_sb[:, t, h], x2_sb[:, t, h], x_sb[:, t, h]
                    )
            for sl in (
                slice(0, 1),
                slice(1, 2),
                slice(2, 3),
                slice(3, 4),
                slice(4, 9),
                slice(9, NCHUNK + 1),
            ):
                nc.scalar.dma_start(out=w_sb[:, sl, :], in_=w_r[:, sl, :])

            basis = [x_sb, x2_sb, x3_sb]
            # 8 PSUM banks: ps[n*4+o] = batch half n, o-tile o, [128, 512] f32
            ps = [
                pspool.tile([128, 512], F32, name=f"ps{i}", tag=f"ps{i}")
                for i in range(2 * PT)
            ]

            def mm(n, o, t, p):
                pe(
                    nc.tensor.matmul(
                        ps[n * PT + o],
                        w_sb[:, 3 * t + p, o * 128 : (o + 1) * 128],
                        basis[p][:, t, n * 512 : (n + 1) * 512],
                        start=(t == 0 and p == 0),
                        stop=(t == PT - 1 and p == NPOW - 1),
                    )
                )

            def copy_out(n, o):
                # PSUM -> SBUF with bias add (f32 -> bf16)
                nc.vector.tensor_scalar_add(
                    y_sb[:, o, n * 512 : (n + 1) * 512],
                    ps[n * PT + o],
                    bias_f32[:, o : o + 1],
                )

            # phase A: batch half 0, t-major (pipelines with x arrival)
            for t in range(PT):
                for p in range(NPOW):
                    for o in range(PT):
                        mm(0, o, t, p)
            # phase B: batch half 1, o-major (early per-o drain).
            # half-0 copies run on DVE as soon as phase A's banks stop.
            for o in range(PT):
                copy_out(0, o)
            for o in range(PT):
                for t in range(PT):
                    for p in range(NPOW):
                        mm(1, o, t, p)
                copy_out(1, o)
                # both halves of o-tile o are in y_sb now; ship it
                queue = nc.scalar if o % 2 == 0 else nc.sync
                queue.dma_start(out=yt_r[:, o, :], in_=y_sb[:, o, :])

    # Post-pass: walrus codegen admits only one sync-wait per instruction
    # encoding here; Tile's kernel-tail drain aggregates one wait per
    # outstanding semaphore. Split any multi-wait instruction into a chain
    # of single-wait drains ahead of it on the same engine queue.
    for bb in nc.m.functions[0].blocks:
        insts = list(bb.instructions)
        out, split = [], 0
        for ins in insts:
            si = ins.sync_info
            waits = list(si.on_wait) if si and si.on_wait else []
            if len(waits) > 1:
                for wx in waits[:-1]:
                    nd = mybir.InstDrain(
                        name=f"drain_split_{split}", engine=ins.engine
                    )
                    split += 1
                    nd.sync_info = mybir.SyncInfo(on_wait=[wx], on_update=[])
                    out.append(nd)
                si.on_wait = [waits[-1]]
            out.append(ins)
        if split:
            bb.set_instructions_from_list(out) if hasattr(
                bb, "set_instructions_from_list"
            ) else setattr(bb, "instructions", out)
    return nc


last_results = None  # BassKernelResults of the most recent run (for test harness)


def kernel(x, edge_w, edge_b, comb_w):
    global _nc, last_results
    if _nc is None:
        _nc = _build()

    bf16 = ml_dtypes.bfloat16
    w_eff = (edge_w * comb_w[:, :, None]).astype(np.float32)  # [I, O, 3]
    # chunk c = 3t+p: rows 128t..128(t+1) of W_p, in matmul consumption order
    w_big = np.empty((NCHUNK + 1, 128, O), dtype=bf16)
    for t in range(PT):
        for p in range(NPOW):
            w_big[3 * t + p] = w_eff[t * 128 : (t + 1) * 128, :, p].astype(bf16)
    # bias chunk: [128,4] f32 bit-packed into bf16 pairs at cols 0..7
    bias = np.sum(comb_w * edge_b, axis=0, dtype=np.float64).astype(np.float32)
    pad = np.zeros((128, O), dtype=bf16)
    pad_u16 = pad.view(np.uint16)
    pad_u16[:, :8] = np.ascontiguousarray(bias.reshape(PT, 128).T).view(np.uint16)
    w_big[NCHUNK] = pad
    # partition-major: [128, 13, 512]
    w_pm = np.ascontiguousarray(w_big.transpose(1, 0, 2))

    in_maps = []
    for c in range(NCORES):
        xs = x[c * BS : (c + 1) * BS].T.astype(bf16)  # [I, BS]
        # partition-major: [128, 4, 1024], [p, t, b] = x^T[t*128+p, b]
        xs_pm = np.ascontiguousarray(xs.reshape(PT, 128, BS).transpose(1, 0, 2))
        in_maps.append({"xt": xs_pm, "w": w_pm})

    res = bass_utils.run_bass_kernel_spmd(_nc, in_maps, list(range(NCORES)))
    last_results = res
    outs = []
    for c in range(NCORES):
        yt = np.asarray(res.results[c]["yt"])  # [128, 4, 1024] bf16
        # un-permute to y^T [512, 1024], then transpose to [1024, 512]
        outs.append(yt.transpose(1, 0, 2).reshape(O, BS).T.astype(np.float32))
    return np.concatenate(outs, axis=0)
